# revision 3
# baseline (speedup 1.0000x reference)
"""3-layer GCN encoder on 8 TRN2 NeuronCores (Bass/Tile).

Math: per layer, out[d] = b + dis[d] * (h'[d] + sum_{e: dst(e)=d} h'[src(e)])
with h' = (x @ W) * dis[:, None], dis = rsqrt(indeg + 1). This folds the
symmetric GCN normalization (norm_e = dis[src]*dis[dst], self-loops included)
into per-node pre/post scaling, so the edge phase is a pure gather + scatter
reduction.

Distribution: nodes row-sharded 6250/core (padded 6272); edges partitioned by
dst core; weights replicated. Per layer each core computes h' for its rows,
an AllGather replicates the full transformed table to every core's HBM, then
each core gathers its edges' source rows (dma_gather, int16 indices via a
lo/hi table split) and scatter-adds them with a one-hot matmul into PSUM
(dst-sorted edge groups of 128; the one-hot selection tile is generated on
the Vector engine by comparing an iota row tile against the per-edge dst-slot
column). Host-side preprocessing is index/layout only (sort, partition,
counts); all float math runs on device.
"""
import os
import sys

sys.path.insert(0, '/opt/trn_rl_repo')

import numpy as np
import ml_dtypes

import concourse.bass as bass
import concourse.mybir as mybir
import concourse.tile as tile
from concourse import bacc
from concourse.alu_op_type import AluOpType
from concourse.bass_utils import run_bass_kernel_spmd
from concourse.masks import make_identity

N_CORES = 8
N_NODES = 50000
NPC = 6250          # nodes per core
NPAD = 6272         # padded (49 * 128)
NB = NPAD // 128    # 49 dst blocks per core
LO_CUT = 4 * NPAD   # table row split for int16 gather indices
CALL = 1024         # gather call size (SWDGE ring limit)
GPC = CALL // 128   # groups per call
AF = mybir.ActivationFunctionType
_last_exec_ns = None

# per layer: (F_in, F_out, table dtype is bf16 unless F_out*2 < 256B)
LAYERS = [(64, 128, True), (128, 128, True), (128, 64, False)]


def _wrap_idx(flat):
    """[n] int -> [128, n/16] SWDGE layout (idx i at [i%16, i//16], x8)."""
    n = len(flat)
    w = np.empty((16, n // 16), dtype=np.int16)
    w[:, :] = flat.astype(np.int16).reshape(n // 16, 16).T
    return np.tile(w, (8, 1))


def _pack(src_row, dst_local, pad_cnt):
    """Pack one core's edges (one half) into a block-aligned gather stream.

    pad_cnt: [NB] uniform (cross-core max) padded edge count per block.
    Returns (gidx_flat int16 [total], slots f32 [total]) with total =
    sum(pad_cnt) rounded up to CALL.
    """
    order = np.lexsort((src_row, dst_local // 128))
    sr = src_row[order]
    dl = dst_local[order]
    bl = dl // 128
    cnt = np.bincount(bl, minlength=NB)
    total = int(pad_cnt.sum())
    total_pad = ((total + CALL - 1) // CALL) * CALL if total else CALL
    g = np.zeros(total_pad, dtype=np.int16)
    s = np.full(total_pad, -1.0, dtype=np.float32)
    starts = np.concatenate([[0], np.cumsum(pad_cnt)])[:-1]
    if cnt.sum():
        pos = starts.repeat(cnt) + np.concatenate(
            [np.arange(c) for c in cnt])
        g[pos] = sr.astype(np.int16)
        s[pos] = (dl - 128 * bl).astype(np.float32)
    return g, s


def _build(ncalls, blk_groups, n_groups):
    """Build the SPMD Bass program.

    ncalls: [2] gather calls per half-stream.
    blk_groups: [2][NB] (g0, g1) group ranges per dst block.
    n_groups: [2] total groups per half-stream.
    """
    nc = bacc.Bacc("TRN2", target_bir_lowering=False, debug=False,
                   num_devices=N_CORES)
    xT_in = nc.dram_tensor("xT", [64, NPAD], mybir.dt.float32, kind="ExternalInput")
    cnt_in = nc.dram_tensor("cnt", [128, NB], mybir.dt.float32, kind="ExternalInput")
    iota_in = nc.dram_tensor("iota", [128, 128], mybir.dt.float32, kind="ExternalInput")
    Ws, Bs = [], []
    for li, (fi, fo, _) in enumerate(LAYERS):
        Ws.append(nc.dram_tensor(f"W{li}", [fi, fo], mybir.dt.float32, kind="ExternalInput"))
        Bs.append(nc.dram_tensor(f"bias{li}", [128, fo], mybir.dt.float32, kind="ExternalInput"))
    gidx_in = [nc.dram_tensor(f"gidx{h}", [128, ncalls[h] * (CALL // 16)],
                              mybir.dt.int16, kind="ExternalInput") for h in (0, 1)]
    slots_in = [nc.dram_tensor(f"slots{h}", [128, n_groups[h]],
                               mybir.dt.float32, kind="ExternalInput") for h in (0, 1)]
    out = nc.dram_tensor("out", [128, NB * 64], mybir.dt.float32, kind="ExternalOutput")

    bounces, tables = [], []
    for li, (fi, fo, is_bf) in enumerate(LAYERS):
        dt = mybir.dt.bfloat16 if is_bf else mybir.dt.float32
        bounces.append(nc.dram_tensor(f"bounce{li}", [NPAD, fo], dt))
        tables.append(nc.dram_tensor(f"table{li}", [NPAD * N_CORES, fo], dt,
                                     addr_space="Shared"))

    with tile.TileContext(nc) as tc:
        with (
            tc.tile_pool(name="const", bufs=1) as constp,
            tc.tile_pool(name="xt", bufs=2) as xtp,
            tc.tile_pool(name="hself", bufs=1) as hsp,
            tc.tile_pool(name="accp", bufs=1) as accp,
            tc.tile_pool(name="rows", bufs=3) as rowsp,
            tc.tile_pool(name="msgs", bufs=4) as msgsp,
            tc.tile_pool(name="idx", bufs=6) as idxp,
            tc.tile_pool(name="oh", bufs=8) as ohp,
            tc.tile_pool(name="mmps", bufs=2, space="PSUM") as mmps,
            tc.tile_pool(name="edgeps", bufs=4, space="PSUM") as edgeps,
            tc.tile_pool(name="trps", bufs=2, space="PSUM") as trps,
        ):
            iota_t = constp.tile([128, 128], mybir.dt.float32)
            nc.sync.dma_start(iota_t[:], iota_in[:])
            ident = constp.tile([128, 128], mybir.dt.float32)
            make_identity(nc, ident[:])
            W_t, B_t = [], []
            for li, (fi, fo, _) in enumerate(LAYERS):
                w = constp.tile([fi, fo], mybir.dt.float32, name=f"w{li}")
                nc.sync.dma_start(w[:], Ws[li][:])
                W_t.append(w)
                b = constp.tile([128, fo], mybir.dt.float32, name=f"b{li}")
                nc.sync.dma_start(b[:], Bs[li][:])
                B_t.append(b)
            slots_t = []
            for h in (0, 1):
                st = constp.tile([128, n_groups[h]], mybir.dt.float32, name=f"slots_t{h}")
                nc.sync.dma_start(st[:], slots_in[h][:])
                slots_t.append(st)
            cnt_t = constp.tile([128, NB], mybir.dt.float32)
            nc.sync.dma_start(cnt_t[:], cnt_in[:])
            dis_t = constp.tile([128, NB], mybir.dt.float32)
            # dis = rsqrt(indeg + 1)  (self-loop included)
            nc.scalar.activation(dis_t[:], cnt_t[:], AF.Sqrt, bias=1.0, scale=1.0)
            nc.vector.reciprocal(dis_t[:], dis_t[:])

            xT = xtp.tile([128, NPAD], mybir.dt.float32, tag="xT")
            nc.sync.dma_start(xT[:64, :], xT_in[:])

            for li, (fi, fo, is_bf) in enumerate(LAYERS):
                tdt = mybir.dt.bfloat16 if is_bf else mybir.dt.float32
                hself = hsp.tile([128, NB, fo], mybir.dt.float32, tag="hself", name=f"hself{li}", padded_shape=[128, NB, 128])
                # h' = (x @ W) * dis ; stage rows to the AllGather bounce
                for t in range(NB):
                    ps = mmps.tile([128, fo], mybir.dt.float32, tag="mm", name=f"mm{li}_{t}")
                    nc.tensor.matmul(ps[:], xT[:fi, t * 128:(t + 1) * 128],
                                     W_t[li][:], start=True, stop=True)
                    nc.vector.tensor_scalar_mul(hself[:, t, :], ps[:],
                                                dis_t[:, t:t + 1])
                    rt = rowsp.tile([128, fo], tdt, tag="rows", name=f"rows{li}_{t}")
                    nc.vector.tensor_copy(rt[:], hself[:, t, :])
                    nc.sync.dma_start(bounces[li][t * 128:(t + 1) * 128, :], rt[:])
                nc.gpsimd.collective_compute(
                    "AllGather", mybir.AluOpType.bypass,
                    replica_groups=[list(range(N_CORES))],
                    ins=[bounces[li].ap().opt()], outs=[tables[li].ap().opt()],
                )

                # edge phase
                acc = accp.tile([128, NB, fo], mybir.dt.float32, tag="acc", name=f"acc{li}", padded_shape=[128, NB, 128])
                msgs_tiles = {}

                def ensure_call(h, c, li=li, fo=fo, tdt=tdt, msgs_tiles=msgs_tiles):
                    if (h, c) in msgs_tiles:
                        return msgs_tiles[(h, c)]
                    gi = idxp.tile([128, CALL // 16], mybir.dt.int16, tag="gi",
                                   name=f"gi{li}_{h}_{c}")
                    nc.sync.dma_start(gi[:], gidx_in[h][:, c * (CALL // 16):(c + 1) * (CALL // 16)])
                    m = msgsp.tile([128, GPC, fo], tdt, tag="msgs",
                                   name=f"m{li}_{h}_{c}")
                    src = tables[li][h * LO_CUT:(h + 1) * LO_CUT, :]
                    nc.gpsimd.dma_gather(m[:], src, gi[:], CALL, CALL, fo)
                    msgs_tiles[(h, c)] = m
                    return m

                for b in range(NB):
                    glist = [(h, g) for h in (0, 1)
                             for g in range(*blk_groups[h][b])]
                    if not glist:
                        nc.vector.tensor_copy(acc[:, b, :], hself[:, b, :])
                        continue
                    ps = edgeps.tile([128, fo], mybir.dt.float32, tag="eps", name=f"eps{li}_{b}")
                    for i, (h, g) in enumerate(glist):
                        m = ensure_call(h, g // GPC)
                        oh = ohp.tile([128, 128], tdt, tag="oh", name=f"oh{li}_{h}_{g}")
                        nc.vector.tensor_scalar(oh[:], iota_t[:],
                                                slots_t[h][:, g:g + 1], None,
                                                op0=AluOpType.is_equal)
                        nc.tensor.matmul(ps[:], oh[:], m[:, g % GPC, :],
                                         start=(i == 0), stop=(i == len(glist) - 1))
                    nc.vector.tensor_tensor(acc[:, b, :], ps[:], hself[:, b, :],
                                            op=AluOpType.add)

                # post: x_next = relu(acc * dis + b); final layer: out rows
                if li < 2:
                    xT = xtp.tile([128, NPAD], mybir.dt.float32, tag="xT",
                                  name=f"xT{li + 1}")
                    for t in range(NB):
                        xr = rowsp.tile([128, fo], mybir.dt.float32, tag="xrow",
                                        name=f"xr{li}_{t}")
                        nc.vector.scalar_tensor_tensor(
                            xr[:], acc[:, t, :], dis_t[:, t:t + 1], B_t[li][:],
                            op0=AluOpType.mult, op1=AluOpType.add)
                        nc.scalar.activation(xr[:], xr[:], AF.Relu)
                        tp = trps.tile([128, 128], mybir.dt.float32, tag="tr", name=f"tr{li}_{t}")
                        nc.tensor.transpose(tp[:], xr[:], ident[:])
                        nc.vector.tensor_copy(xT[:, t * 128:(t + 1) * 128], tp[:])
                else:
                    orows = rowsp.tile([128, NB, 64], mybir.dt.float32, tag="orows")
                    for t in range(NB):
                        nc.vector.scalar_tensor_tensor(
                            orows[:, t, :], acc[:, t, :], dis_t[:, t:t + 1],
                            B_t[li][:, :64], op0=AluOpType.mult, op1=AluOpType.add)
                    nc.sync.dma_start(out[:, :], orows.rearrange("p b f -> p (b f)"))
    nc.compile()
    return nc


def kernel(x, edge_index, W1, b1, W2, b2, W3, b3):
    global _last_exec_ns
    x = np.asarray(x, dtype=np.float32)
    edge_index = np.asarray(edge_index)
    Ws = [np.asarray(w, dtype=np.float32) for w in (W1, W2, W3)]
    bs = [np.asarray(b, dtype=np.float32) for b in (b1, b2, b3)]

    src = edge_index[0].astype(np.int64)
    dst = edge_index[1].astype(np.int64)
    src_row = (src // NPC) * NPAD + src % NPC
    dst_core = dst // NPC
    dst_local = dst % NPC
    half = (src_row >= LO_CUT).astype(np.int64)

    # uniform cross-core schedule: padded group counts per (half, block)
    pad_cnt = []
    per_core = []
    for h in (0, 1):
        cnts = np.zeros((N_CORES, NB), dtype=np.int64)
        for c in range(N_CORES):
            m = (dst_core == c) & (half == h)
            cnts[c] = np.bincount(dst_local[m] // 128, minlength=NB)
        pad_cnt.append(((cnts.max(axis=0) + 127) // 128) * 128)
    ncalls, n_groups, blk_groups = [], [], []
    for h in (0, 1):
        total = int(pad_cnt[h].sum())
        total_pad = ((total + CALL - 1) // CALL) * CALL if total else CALL
        ncalls.append(total_pad // CALL)
        n_groups.append(total_pad // 128)
        starts = np.concatenate([[0], np.cumsum(pad_cnt[h])])[:-1]
        blk_groups.append([(int(starts[b] // 128),
                            int((starts[b] + pad_cnt[h][b]) // 128))
                           for b in range(NB)])

    nc = _build(ncalls, blk_groups, n_groups)

    iota = np.tile(np.arange(128, dtype=np.float32), (128, 1))
    in_maps = []
    for c in range(N_CORES):
        xc = np.zeros((64, NPAD), dtype=np.float32)
        xc[:, :NPC] = x[c * NPC:(c + 1) * NPC].T
        mm = dst_core == c
        cnt_flat = np.bincount(dst_local[mm], minlength=NPAD).astype(np.float32)
        im = {
            "xT": xc,
            "cnt": cnt_flat.reshape(NB, 128).T.copy(),
            "iota": iota,
        }
        for li in range(3):
            im[f"W{li}"] = Ws[li]
            im[f"bias{li}"] = np.tile(bs[li], (128, 1)).astype(np.float32)
        for h in (0, 1):
            sel = mm & (half == h)
            g, s = _pack(src_row[sel] - h * LO_CUT, dst_local[sel], pad_cnt[h])
            im[f"gidx{h}"] = _wrap_idx(g)
            im[f"slots{h}"] = s.reshape(-1, 128).T.copy()
        in_maps.append(im)

    trace = os.environ.get("KERNEL_TRACE", "0") == "1"
    res = run_bass_kernel_spmd(nc, in_maps, core_ids=list(range(N_CORES)),
                               trace=trace)
    _last_exec_ns = res.exec_time_ns

    outp = np.empty((N_NODES, 64), dtype=np.float32)
    for c in range(N_CORES):
        o = res.results[c]["out"].reshape(128, NB, 64).transpose(1, 0, 2)
        outp[c * NPC:(c + 1) * NPC] = o.reshape(NPAD, 64)[:NPC]
    return outp
